# revision 2
# baseline (speedup 1.0000x reference)
"""Mixture-of-Depths block kernel v2 for 8 TRN2 NeuronCores (Bass/Tile).

Data-parallel over batch B=8, one batch row per core. v2 replaces v1's
indirect-DMA gather/scatter through a DRAM compact buffer with one-hot
matmul gather/scatter on the tensor engine:

  S0  stream x in 128-token chunks; exact-fp32 router logits (DVE+Pool),
      16-ary threshold search, prefix-sum slot positions o_f (>=K for
      unselected tokens), router weights w_tok, and a DRAM staging row
      of (o_f, w_tok) for the scatter one-hot.
  S1  one-hot P[t,k]=(o_f[t]==k) via is_equal; x cast to bf16; gather
      matmul cb[k,:] = P^T x (eviction fused with LN1 row-sum); scatter
      one-hot Pw[k,t]=w[t]*(o_f[t]==k) built from the broadcast staging
      row (fills DVE/Pool slack under the gather matmul).
  S2  LN1 (fused sumsq, stats on [128,8] vectors) -> anorm -> aT.
  S3  QKV.  S4 causal attention (ones-row V trick for denominators).
  S5  o_proj WITHOUT residual (dsb) + h2 = dsb + cb in-place (fused
      LN2 row-sum).  S6 LN2 -> mT.  S7 fc + gelu -> gT.  S8 proj +
      delta = proj_out + dsb.  S9 scatter matmul out_tok = Pw^T delta,
      combined as out = x*(1 + w*sel) + scatter, streamed per t-chunk.

out[t] = x[t]*(1+w[t]*sel[t]) + sum_k Pw[k,t]*(attn_out+mlp)[k] equals
the reference's x.at[idx].add(w * processed) because processed =
tokens + attn_out + mlp and the tokens term collapses to w[t]*x[t].
"""
import numpy as np
import ml_dtypes

import concourse.bass as bass
import concourse.mybir as mybir
import concourse.tile as tile
from concourse.bass_utils import run_bass_kernel_spmd
from concourse.vector_clock import ScopedClock, VectorClock

dt = mybir.dt
Alu = mybir.AluOpType
Act = mybir.ActivationFunctionType

MAX_WAITS = 1


def fix_sync_waits(nc, max_waits=MAX_WAITS):
    n_split = 0
    for f in nc.m.functions:
        for bb in f.blocks:
            new = []
            for inst in bb.instructions:
                si = inst.sync_info
                if si is not None and si.on_wait and len(si.on_wait) > max_waits:
                    waits = list(si.on_wait)
                    extra, keep = waits[:-max_waits], waits[-max_waits:]
                    for w in extra:
                        n_split += 1
                        nop = mybir.InstNoOp(name=f"{inst.name}-ws{n_split}")
                        nop.engine = inst.engine
                        nop.sync_info = mybir.SyncInfo(on_wait=[w], on_update=[])
                        new.append(nop)
                    inst.sync_info = mybir.SyncInfo(
                        on_wait=keep, on_update=list(si.on_update))
                new.append(inst)
            bb.instructions[:] = new
    return n_split


class FunnelTileContext(tile.TileContext):
    """TileContext whose tail drain's waits are split across funnel drains."""

    def _drain_and_barrier(self, tick_clock, wait_clock):
        gc = tick_clock.global_clock
        ticks = eval(repr(gc).replace('VectorClock(', '').rstrip(')'))
        for i, t in enumerate(ticks):
            if t > 0:
                partial = [0] * 27
                partial[i] = t
                d = self.nc.sync.drain()
                wait_clock.add_sem_waits(d.ins, ScopedClock({None: VectorClock(partial)}))
        self.nc.sync.drain()
        self.nc.all_engine_barrier()
        assert self.sems is not None
        popped = self.nc._tile_sem_poison_stack.pop()
        assert popped is self._sem_poison
        sems = list(self.sems.allocated().values())
        for i in range(0, len(sems), 8):
            self.nc.clear_and_free_semaphores(sems[i:i + 8])
        self.nc.all_engine_barrier()


B, T, C = 8, 2048, 1024
K = 1024
H = 16
DH = C // H
DFF = 4 * C
EPS = 1e-5
NCH = T // 128    # 16
NQ = K // 128     # 8
SRCH_ITERS = 7
LO0, STEP0 = -8.0, 1.0

F32, BF16, I32, I16 = dt.float32, dt.bfloat16, dt.int32, dt.int16
FP8 = dt.float8e4
FP8_QKV = True      # fp8 DoubleRow QKV projection (error-budget tested)
FP8_FC = False      # fc/proj stay bf16: fp8 there breaks the 2e-2 budget
FP8_PROJ = False
WSCALE = 16.0       # fp8 weight prescale (descaled at PSUM eviction)


def host_inputs(inputs):
    x = np.asarray(inputs["x"], np.float32)
    assert x.shape == (B, T, C)
    assert int(inputs["top_k"]) == K and int(inputs["n_head"]) == H

    def bf(a):
        return np.ascontiguousarray(np.asarray(a, np.float32)).astype(ml_dtypes.bfloat16)

    def f8(aT, pairs):
        # [R, F] -> [R/256, 2, 128, F] fp8 with rows prescaled by WSCALE
        a = np.ascontiguousarray(np.asarray(aT, np.float32)) * WSCALE
        R, F = a.shape
        return a.reshape(R // 256, 2, 128, F).astype(ml_dtypes.float8_e4m3fn)

    common = {
        "wrt128": np.ascontiguousarray(np.broadcast_to(
            np.asarray(inputs["w_router"], np.float32), (128, C))),
        "woT": bf(np.asarray(inputs["w_o"], np.float32).T),
        "stair": bf(np.triu(np.ones((128, 128), np.float32))),
        "iota15": np.ascontiguousarray(np.broadcast_to(
            np.arange(1, 16, dtype=np.float32), (128, 15))),
        "iotaT": np.ascontiguousarray(
            np.arange(T, dtype=np.float32).reshape(NCH, 128).T),
        "utri": np.triu(np.ones((128, 128), np.float32), 1),
        "ones2d": np.ones((128, 128), np.float32),
        "onesbf": bf(np.ones((128, 128), np.float32)),
        "ident_bf": bf(np.eye(128, dtype=np.float32)),
        "iotaK": np.ascontiguousarray(np.broadcast_to(
            np.arange(K, dtype=np.int16), (128, K))),
    }
    if FP8_QKV:
        common["wqkvT8"] = f8(np.asarray(inputs["w_qkv"], np.float32).T, 2)
    else:
        common["wqkvT"] = bf(np.asarray(inputs["w_qkv"], np.float32).T)
    if FP8_FC:
        common["wfcT8"] = f8(np.asarray(inputs["w_fc"], np.float32).T, 2)
    else:
        common["wfcT"] = bf(np.asarray(inputs["w_fc"], np.float32).T)
    if FP8_PROJ:
        common["wprojT8"] = f8(np.asarray(inputs["w_proj"], np.float32).T, 2)
    else:
        common["wprojT"] = bf(np.asarray(inputs["w_proj"], np.float32).T)
    for nm in ("ln1_w", "ln2_w"):
        assert np.all(np.asarray(inputs[nm]) == 1), nm
    for nm in ("ln1_b", "ln2_b", "b_qkv", "b_o", "b_fc", "b_proj"):
        assert np.all(np.asarray(inputs[nm]) == 0), nm

    return [dict(common, xb=np.ascontiguousarray(x[b])) for b in range(B)]


def declare_io(nc, dbg_names=()):
    io = {}
    io["xb"] = nc.dram_tensor("xb", [T, C], F32, kind="ExternalInput")
    io["wrt128"] = nc.dram_tensor("wrt128", [128, C], F32, kind="ExternalInput")
    io["woT"] = nc.dram_tensor("woT", [C, C], BF16, kind="ExternalInput")
    if FP8_QKV:
        io["wqkvT8"] = nc.dram_tensor("wqkvT8", [C // 256, 2, 128, 3 * C], FP8,
                                      kind="ExternalInput")
    else:
        io["wqkvT"] = nc.dram_tensor("wqkvT", [C, 3 * C], BF16, kind="ExternalInput")
    if FP8_FC:
        io["wfcT8"] = nc.dram_tensor("wfcT8", [C // 256, 2, 128, DFF], FP8,
                                     kind="ExternalInput")
    else:
        io["wfcT"] = nc.dram_tensor("wfcT", [C, DFF], BF16, kind="ExternalInput")
    if FP8_PROJ:
        io["wprojT8"] = nc.dram_tensor("wprojT8", [DFF // 256, 2, 128, C], FP8,
                                       kind="ExternalInput")
    else:
        io["wprojT"] = nc.dram_tensor("wprojT", [DFF, C], BF16, kind="ExternalInput")
    io["stair"] = nc.dram_tensor("stair", [128, 128], BF16, kind="ExternalInput")
    io["iota15"] = nc.dram_tensor("iota15", [128, 15], F32, kind="ExternalInput")
    io["iotaT"] = nc.dram_tensor("iotaT", [128, NCH], F32, kind="ExternalInput")
    io["utri"] = nc.dram_tensor("utri", [128, 128], F32, kind="ExternalInput")
    io["ones2d"] = nc.dram_tensor("ones2d", [128, 128], F32, kind="ExternalInput")
    io["onesbf"] = nc.dram_tensor("onesbf", [128, 128], BF16, kind="ExternalInput")
    io["ident_bf"] = nc.dram_tensor("ident_bf", [128, 128], BF16, kind="ExternalInput")
    io["iotaK"] = nc.dram_tensor("iotaK", [128, K], I16, kind="ExternalInput")
    io["out"] = nc.dram_tensor("out", [T, C], F32, kind="ExternalOutput")
    io["rowscr"] = nc.dram_tensor("rowscr", [2, T], F32, kind="Internal")
    dbg = {}
    shapes = {"o_f": ([128, NCH], F32), "ls": ([128, NCH], F32),
              "lo": ([128, 1], F32), "cb": ([128, NQ, C], BF16),
              "anorm": ([128, NQ, C], BF16), "aT": ([128, NQ, K], BF16),
              "qk": ([128, 2 * NQ, K], BF16), "attnT": ([128, NQ, K], BF16),
              "hsb": ([128, NQ, C], BF16), "gT": ([128, DFF // 128, K], BF16),
              "delta": ([128, NQ, C], BF16), "pw": ([128, NQ, T], BF16)}
    for nm in dbg_names:
        sh, d = shapes[nm]
        dbg[nm] = nc.dram_tensor("dbg_" + nm, sh, d, kind="ExternalOutput")
    return io, dbg


def build(nc, tc, io, dbg=None, last_stage=99):
    opened = []
    try:
        _build(nc, tc, io, dbg or {}, last_stage, opened)
    finally:
        for p in reversed(opened):
            p._cm.__exit__(None, None, None)


def _build(nc, tc, io, dbg, last_stage, opened):
    def pool(name, bufs, space=None, side="left"):
        kw = {"space": space} if space else {}
        if not space:
            kw["side"] = side
        cm = tc.tile_pool(name=name, bufs=bufs, **kw)
        p = cm.__enter__()
        p._cm = cm
        p._side = kw.get("side", "psum")
        opened.append(p)
        return p

    def close(*ps):
        for p in sorted(ps, key=opened.index, reverse=True):
            same = [q for q in opened if q._side == p._side]
            assert same[-1] is p, (p.name, [q.name for q in opened])
            opened.remove(p)
            p._cm.__exit__(None, None, None)

    xb, out = io["xb"].ap(), io["out"].ap()
    rowscr = io["rowscr"].ap()
    xbr = xb.rearrange("(c p) d -> p c d", p=128)

    def dump(nm, ap_or_tile):
        if nm in dbg:
            nc.sync.dma_start(out=dbg[nm].ap(), in_=ap_or_tile)

    cpool = pool("const", 1)
    consts = {}
    for nm, shape, d in (("wrt128", [128, C], F32), ("stair", [128, 128], BF16),
                         ("iota15", [128, 15], F32), ("iotaT", [128, NCH], F32),
                         ("utri", [128, 128], F32), ("ones2d", [128, 128], F32),
                         ("onesbf", [128, 128], BF16), ("ident_bf", [128, 128], BF16),
                         ("iotaK", [128, K], I16)):
        t = cpool.tile(shape, d, name="c_" + nm)
        nc.sync.dma_start(out=t[:], in_=io[nm].ap())
        consts[nm] = t
    wrt, stair, iota15, iotaT = (consts["wrt128"], consts["stair"],
                                 consts["iota15"], consts["iotaT"])
    utri, ones2d, onesbf, ident = (consts["utri"], consts["ones2d"],
                                   consts["onesbf"], consts["ident_bf"])
    iotaK = consts["iotaK"]

    # long-lived small state
    rpool = pool("router", 1)
    epsc = rpool.tile([128, 1], F32)
    nc.vector.memset(epsc[:], EPS)
    pofs_i = rpool.tile([128, NQ], I32)
    nc.gpsimd.iota(pofs_i[:], pattern=[[128, NQ]], base=0, channel_multiplier=1)
    pofs = rpool.tile([128, NQ], F32)
    nc.gpsimd.tensor_copy(pofs[:], pofs_i[:])
    junk = rpool.tile([128, C], F32, name="junk")

    # cb: gathered tokens, then h2 in-place; lives S1..end of fc phase
    cbp = pool("cb", 1)
    cb = cbp.tile([128, NQ, C], BF16)
    stats = rpool.tile([128, NQ], F32, name="ssum8")
    ssq8 = rpool.tile([128, NQ], F32, name="ssq8")

    # ---------------- S0: stream x chunks, router, top-k ------------------
    ppool = pool("ponehot", 1)
    P = ppool.tile([128, NCH, K], BF16)
    xbfp = pool("xbf", 1)
    xbf = xbfp.tile([128, NCH, C], BF16)
    xsp = pool("xs", 6)
    ls = rpool.tile([128, NCH], F32)
    for c in range(NCH):
        xsc = xsp.tile([128, C], F32, tag="xsc")
        nc.sync.dma_start(out=xsc[:], in_=xbr[:, c, :])
        # exact-fp32 router logits (fused multiply+reduce on DVE)
        nc.vector.scalar_tensor_tensor(
            out=junk[:], in0=xsc[:], scalar=1.0, in1=wrt[:],
            op0=Alu.mult, op1=Alu.mult, accum_out=ls[:, c:c + 1])
        # bf16 cast for the gather matmul (Activation engine, idle here)
        nc.scalar.copy(out=xbf[:, c, :], in_=xsc[:])
    close(xsp)

    lo = rpool.tile([128, 1], F32)
    step = rpool.tile([128, 1], F32)
    nc.vector.memset(lo[:], LO0)
    nc.vector.memset(step[:], STEP0)
    mids = rpool.tile([128, 15], F32)
    cmp3 = rpool.tile([128, 15, NCH], F32)
    red = rpool.tile([128, 15], F32)
    scrap = rpool.tile([128, 15], F32)
    nbuk = rpool.tile([128, 1], F32)
    psum_srch = pool("psum_srch", 2, "PSUM")
    for it in range(SRCH_ITERS):
        nc.vector.scalar_tensor_tensor(
            out=mids[:], in0=iota15[:], scalar=step[:, 0:1],
            in1=lo[:, 0:1].to_broadcast([128, 15]), op0=Alu.mult, op1=Alu.add)
        nc.vector.tensor_tensor(
            out=cmp3[:], in0=ls[:].unsqueeze(1).to_broadcast([128, 15, NCH]),
            in1=mids[:].unsqueeze(2).to_broadcast([128, 15, NCH]), op=Alu.is_gt)
        nc.vector.tensor_reduce(out=red[:], in_=cmp3[:], axis=mybir.AxisListType.X,
                                op=Alu.add)
        cnt = psum_srch.tile([128, 15], F32, tag="cnt")
        nc.tensor.matmul(out=cnt[:], lhsT=ones2d[:], rhs=red[:], start=True, stop=True)
        nc.vector.tensor_scalar(out=scrap[:], in0=cnt[:], scalar1=float(K),
                                scalar2=None, op0=Alu.is_ge, op1=Alu.add,
                                accum_out=nbuk[:])
        nc.vector.scalar_tensor_tensor(out=lo[:], in0=nbuk[:], scalar=step[:, 0:1],
                                       in1=lo[:], op0=Alu.mult, op1=Alu.add)
        nc.vector.tensor_scalar_mul(step[:], step[:], 1.0 / 16.0)

    mask = rpool.tile([128, NCH], F32)
    nc.vector.tensor_scalar(out=mask[:], in0=ls[:], scalar1=lo[:, 0:1],
                            scalar2=None, op0=Alu.is_gt)
    pre = psum_srch.tile([128, NCH], F32, tag="pre")
    nc.tensor.matmul(out=pre[:], lhsT=utri[:], rhs=mask[:], start=True, stop=True)
    tot = psum_srch.tile([128, NCH], F32, tag="tot")
    nc.tensor.matmul(out=tot[:], lhsT=ones2d[:], rhs=mask[:], start=True, stop=True)
    ex = rpool.tile([128, NCH], F32)
    ex2 = rpool.tile([128, NCH], F32)
    nc.vector.memset(ex[:, 0:1], 0.0)
    nc.vector.tensor_copy(ex[:, 1:NCH], tot[:, 0:NCH - 1])
    cur, nxt = ex, ex2
    for d in (1, 2, 4, 8):
        nc.vector.tensor_copy(nxt[:, 0:d], cur[:, 0:d])
        nc.vector.tensor_tensor(out=nxt[:, d:NCH], in0=cur[:, d:NCH],
                                in1=cur[:, 0:NCH - d], op=Alu.add)
        cur, nxt = nxt, cur
    pos = rpool.tile([128, NCH], F32)
    nc.vector.tensor_tensor(out=pos[:], in0=pre[:], in1=cur[:], op=Alu.add)
    alt = rpool.tile([128, NCH], F32)
    nc.vector.scalar_tensor_tensor(out=alt[:], in0=iotaT[:], scalar=float(K),
                                   in1=pos[:], op0=Alu.add, op1=Alu.subtract)
    dif = rpool.tile([128, NCH], F32)
    nc.vector.tensor_tensor(out=dif[:], in0=pos[:], in1=alt[:], op=Alu.subtract)
    nc.vector.tensor_tensor(out=dif[:], in0=dif[:], in1=mask[:], op=Alu.mult)
    o_f = rpool.tile([128, NCH], F32)
    nc.vector.tensor_tensor(out=o_f[:], in0=alt[:], in1=dif[:], op=Alu.add)
    w_tok = rpool.tile([128, NCH], F32)
    nc.vector.tensor_tensor(out=w_tok[:], in0=ls[:], in1=mask[:], op=Alu.mult)
    wm1 = rpool.tile([128, NCH], F32)
    nc.vector.tensor_scalar(out=wm1[:], in0=w_tok[:], scalar1=1.0,
                            scalar2=None, op0=Alu.add)
    nc.sync.dma_start(out=rowscr[0, :].rearrange("(c p) -> p c", p=128),
                      in_=o_f[:])
    nc.sync.dma_start(out=rowscr[1, :].rearrange("(c p) -> p c", p=128),
                      in_=w_tok[:])
    close(psum_srch)

    dump("o_f", o_f[:])
    dump("ls", ls[:])
    dump("lo", lo[:])
    if last_stage < 1:
        return

    # ---------------- S1: gather one-hot + gather matmul ------------------
    # right-side pools: pw (scatter one-hot, lives to S9), wq (QKV weights,
    # prefetched now), bcb (broadcast o_f/w rows, transient)
    pwp = pool("pw", 1, side="right")
    pw = pwp.tile([128, NQ, T], BF16)
    if FP8_QKV:
        wqp = pool("wqkv", 1, side="right")
        wq = wqp.tile([128, C // 256, 2, 3 * C], FP8)
        nc.sync.dma_start(out=wq[:],
                          in_=io["wqkvT8"].ap().rearrange("c i p f -> p c i f"))
    else:
        wqp = pool("wqkv", 1, side="right")
        wq = wqp.tile([128, NQ, 3 * C], BF16)
        nc.sync.dma_start(out=wq[:],
                          in_=io["wqkvT"].ap().rearrange("(cc p) f -> p cc f", p=128))
    bcbp = pool("bcb", 1, side="right")
    bcb = bcbp.tile([128, 2, T], F32)
    nc.sync.dma_start(out=bcb[:],
                      in_=rowscr.unsqueeze(0).to_broadcast([128, 2, T]))

    for c in range(NCH):
        nc.vector.tensor_scalar(out=P[:, c, :], in0=iotaK[:],
                                scalar1=o_f[:, c:c + 1],
                                scalar2=None, op0=Alu.is_equal)

    psum_g = pool("psum_g", 2, "PSUM")
    for kc in range(NQ):
        pg = psum_g.tile([128, C], F32, tag="pg")
        for tc2 in range(2):
            for c in range(NCH):
                nc.tensor.matmul(out=pg[:, tc2 * 512:(tc2 + 1) * 512],
                                 lhsT=P[:, c, kc * 128:(kc + 1) * 128],
                                 rhs=xbf[:, c, tc2 * 512:(tc2 + 1) * 512],
                                 start=(c == 0), stop=(c == NCH - 1))
        nc.vector.tensor_copy(cb[:, kc, :], pg[:])
        nc.vector.tensor_reduce(out=stats[:, kc:kc + 1], in_=pg[:],
                                axis=mybir.AxisListType.X, op=Alu.add)
        nc.vector.scalar_tensor_tensor(
            out=junk[:], in0=cb[:, kc, :], scalar=1.0, in1=cb[:, kc, :],
            op0=Alu.mult, op1=Alu.mult, accum_out=ssq8[:, kc:kc + 1])
        # scatter one-hot build rides the DVE slack under the gather matmul
        nc.vector.scalar_tensor_tensor(out=pw[:, kc, :], in0=bcb[:, 0, :],
                                       scalar=pofs[:, kc:kc + 1], in1=bcb[:, 1, :],
                                       op0=Alu.is_equal, op1=Alu.mult)
    close(psum_g, xbfp, ppool, bcbp)
    dump("pw", pw[:])
    dump("cb", cb[:])
    if last_stage < 2:
        return

    # ---------------- S2: LN1 + transpose to aT ---------------------------
    lnp = pool("ln", 2)

    def ln_stats(ssum8, sq8, mu8, rstd8):
        nc.vector.tensor_scalar_mul(mu8[:], ssum8[:], 1.0 / C)
        nmu2 = lnp.tile([128, NQ], F32, tag="nmu2")
        nc.vector.tensor_tensor(out=nmu2[:], in0=mu8[:], in1=mu8[:], op=Alu.mult)
        var8 = lnp.tile([128, NQ], F32, tag="var8")
        nc.vector.scalar_tensor_tensor(out=var8[:], in0=sq8[:], scalar=1.0 / C,
                                       in1=nmu2[:], op0=Alu.mult, op1=Alu.subtract)
        lgv = lnp.tile([128, NQ], F32, tag="lgv")
        nc.scalar.activation(out=lgv[:], in_=var8[:], func=Act.Ln, bias=epsc[:, 0:1])
        nc.scalar.activation(out=rstd8[:], in_=lgv[:], func=Act.Exp, scale=-0.5)

    mu8 = rpool.tile([128, NQ], F32, name="mu8")
    rstd8 = rpool.tile([128, NQ], F32, name="rstd8")
    ln_stats(stats, ssq8, mu8, rstd8)

    qkp = pool("qk", 1)
    qk = qkp.tile([128, 2 * NQ, K], BF16)
    vbp = pool("vb", 1)
    vb = vbp.tile([128, NQ, H * (DH + 1)], BF16)
    atp = pool("aT", 1)
    aT = atp.tile([128, NQ, K], FP8 if FP8_QKV else BF16)
    anp = pool("anorm", 1)
    anorm = anp.tile([128, NQ, C], BF16)
    for kc in range(NQ):
        nc.vector.tensor_scalar(out=anorm[:, kc, :], in0=cb[:, kc, :],
                                scalar1=mu8[:, kc:kc + 1], scalar2=rstd8[:, kc:kc + 1],
                                op0=Alu.subtract, op1=Alu.mult)
    dump("anorm", anorm[:])

    def transpose_block(src3, dst3, n_row, n_col, tp):
        # j2 outer so low c-tiles complete first (consumers read c-pairs
        # across all k); evictions alternate DVE/Act to halve the chain.
        for j2 in range(0, n_col, 4):
            jm = min(j2 + 4, n_col)
            for i in range(n_row):
                pt = tp.tile([128, 512], BF16, tag="pt")
                for j in range(j2, jm):
                    nc.tensor.transpose(out=pt[:, (j - j2) * 128:(j - j2 + 1) * 128],
                                        in_=src3[:, i, j * 128:(j + 1) * 128],
                                        identity=ident[:])
                dst = dst3[:, j2:jm, i * 128:(i + 1) * 128]
                src = pt[:, 0:(jm - j2) * 128].rearrange("p (j d) -> p j d", d=128)
                if i % 2 == 0:
                    nc.scalar.copy(out=dst, in_=src)
                else:
                    nc.vector.tensor_copy(dst, src)

    ptp1 = pool("psum_t1", 4, "PSUM")
    transpose_block(anorm, aT, NQ, NQ, ptp1)
    close(ptp1, anp)
    dump("aT", aT[:])
    if last_stage < 3:
        return

    # ---------------- S3: QKV -------------------------------------------
    if FP8_QKV:
        pqk = pool("psum_qk", 4, "PSUM")
        DR = mybir.MatmulPerfMode.DoubleRow
        for mf in range(2 * NQ):
            for nt in range(2):
                ps = pqk.tile([128, 512], F32, tag="ps")
                for c2 in range(C // 256):
                    nc.tensor.matmul(out=ps[:],
                                     lhsT=wq[:, c2, :, mf * 128:(mf + 1) * 128],
                                     rhs=aT[:, 2 * c2:2 * c2 + 2,
                                            nt * 512:(nt + 1) * 512],
                                     start=(c2 == 0), stop=(c2 == C // 256 - 1),
                                     perf_mode=DR)
                nc.vector.tensor_scalar(out=qk[:, mf, nt * 512:(nt + 1) * 512],
                                        in0=ps[:], scalar1=1.0 / WSCALE,
                                        scalar2=None, op0=Alu.mult)
        for tt in range(NQ):
            for nt in range(2):
                ps = pqk.tile([128, 512], F32, tag="ps")
                for c2 in range(C // 256):
                    nc.tensor.matmul(
                        out=ps[:],
                        lhsT=aT[:, 2 * c2:2 * c2 + 2, tt * 128:(tt + 1) * 128],
                        rhs=wq[:, c2, :, 2 * C + nt * 512:2 * C + (nt + 1) * 512],
                        start=(c2 == 0), stop=(c2 == C // 256 - 1), perf_mode=DR)
                dst = vb[:, tt, :].rearrange("p (h d) -> p h d", d=DH + 1)
                nc.vector.tensor_scalar(
                    out=dst[:, nt * 8:(nt + 1) * 8, 0:DH],
                    in0=ps[:].rearrange("p (h d) -> p h d", d=DH),
                    scalar1=1.0 / WSCALE, scalar2=None, op0=Alu.mult)
    else:
        pqk = pool("psum_qk", 4, "PSUM")
        for mf in range(2 * NQ):
            for nt in range(2):
                ps = pqk.tile([128, 512], F32, tag="ps")
                for cc in range(NQ):
                    nc.tensor.matmul(out=ps[:], lhsT=wq[:, cc, mf * 128:(mf + 1) * 128],
                                     rhs=aT[:, cc, nt * 512:(nt + 1) * 512],
                                     start=(cc == 0), stop=(cc == NQ - 1))
                nc.vector.tensor_copy(qk[:, mf, nt * 512:(nt + 1) * 512], ps[:])
        for tt in range(NQ):
            for nt in range(2):
                ps = pqk.tile([128, 512], F32, tag="ps")
                for cc in range(NQ):
                    nc.tensor.matmul(out=ps[:], lhsT=aT[:, cc, tt * 128:(tt + 1) * 128],
                                     rhs=wq[:, cc, 2 * C + nt * 512:2 * C + (nt + 1) * 512],
                                     start=(cc == 0), stop=(cc == NQ - 1))
                dst = vb[:, tt, :].rearrange("p (h d) -> p h d", d=DH + 1)
                nc.vector.tensor_copy(dst[:, nt * 8:(nt + 1) * 8, 0:DH],
                                      ps[:].rearrange("p (h d) -> p h d", d=DH))
    ones_col = vb[:].rearrange("p q (h d) -> p q h d", d=DH + 1)[:, :, :, DH:DH + 1]
    nc.vector.memset(ones_col, 1.0)
    close(pqk, wqp, atp)
    dump("qk", qk[:])
    if last_stage < 4:
        return

    # ---------------- S4: attention --------------------------------------
    ssum2 = rpool.tile([128, NQ], F32, name="ssum2")
    ssq2 = rpool.tile([128, NQ], F32, name="ssq2")
    wop = pool("wo", 1)
    wo = wop.tile([128, NQ, C], BF16)
    nc.sync.dma_start(out=wo[:], in_=io["woT"].ap().rearrange("(cc p) f -> p cc f", p=128))
    att_p = pool("attnT", 1)
    attnT = att_p.tile([128, NQ, K], BF16)
    den_p = pool("den", 1)
    den_sb = den_p.tile([128, NQ, K], BF16)
    nump = pool("num", 1)
    ps_s = pool("psum_s", 2, "PSUM")
    ps_a = pool("psum_a", 2, "PSUM")
    rowp = pool("denrow", 2)

    for j in range(H // 2):
        nums = []
        for hh in range(2):
            h = 2 * j + hh
            p0 = 64 * hh
            num = nump.tile([128, NQ, K], BF16, tag=f"num{hh}")
            nums.append(num)
            mfK = NQ + j
            for kc in range(NQ):
                qlo = kc * 128
                ps = ps_s.tile([128, 1024], F32, tag="ps_s")
                for q0, q1 in ((qlo, min(qlo + 512, K)), (qlo + 512, K)):
                    if q1 <= q0:
                        continue
                    nc.tensor.matmul(
                        out=ps[:, q0 - qlo:q1 - qlo],
                        lhsT=qk[p0:p0 + DH, mfK, kc * 128:(kc + 1) * 128],
                        rhs=qk[p0:p0 + DH, j, q0:q1],
                        start=True, stop=True)
                nc.scalar.activation(out=num[:, kc, qlo:K],
                                     in_=ps[:, 0:K - qlo], func=Act.Exp,
                                     scale=0.125)
                dg = kc * 128
                nc.vector.tensor_tensor(out=num[:, kc, dg:dg + 128],
                                        in0=num[:, kc, dg:dg + 128],
                                        in1=stair[:], op=Alu.mult)
        for hh in range(2):
            h = 2 * j + hh
            num = nums[hh]
            for nt in range(2):
                pa = ps_a.tile([128, 512], F32, tag="ps_a")
                for qt in range(4 * nt, 4 * nt + 4):
                    co = qt * 128 - nt * 512
                    for kc in range(qt + 1):
                        nc.tensor.matmul(
                            out=pa[0:DH + 1, co:co + 128],
                            lhsT=vb[:, kc, h * (DH + 1):(h + 1) * (DH + 1)],
                            rhs=num[:, kc, qt * 128:(qt + 1) * 128],
                            start=(kc == 0), stop=(kc == qt))
                nc.vector.tensor_copy(
                    attnT[64 * hh:64 * hh + 64, j, nt * 512:(nt + 1) * 512],
                    pa[0:DH, :])
                drow = rowp.tile([128, 512], BF16, tag="drow")
                nc.vector.tensor_copy(drow[64:65, :], pa[DH:DH + 1, :])
                pd = ps_a.tile([128, 512], F32, tag="pd")
                nc.tensor.matmul(out=pd[0:64, :], lhsT=onesbf[64:65, 0:64],
                                 rhs=drow[64:65, :], start=True, stop=True)
                nc.vector.tensor_copy(
                    den_sb[64 * hh:64 * hh + 64, j, nt * 512:(nt + 1) * 512],
                    pd[0:64, :])
    close(rowp, ps_a, ps_s, nump)
    recp = pool("rec", 2)
    for cm in range(NQ):
        for nh in range(2):
            hsl = slice(nh * 512, (nh + 1) * 512)
            lgd = recp.tile([128, 512], F32, tag="lgd")
            nc.scalar.activation(out=lgd[:], in_=den_sb[:, cm, hsl], func=Act.Ln)
            rec = recp.tile([128, 512], BF16, tag="rec")
            nc.scalar.activation(out=rec[:], in_=lgd[:], func=Act.Exp, scale=-1.0)
            nc.vector.tensor_tensor(out=attnT[:, cm, hsl], in0=attnT[:, cm, hsl],
                                    in1=rec[:], op=Alu.mult)
    close(recp, den_p)
    dump("attnT", attnT[:])
    if last_stage < 5:
        return

    # ---------------- S5: o_proj (no residual) + h2 in-place --------------
    dsp = pool("dsb", 1, side="right")
    dsb = dsp.tile([128, NQ, C], BF16)
    pso = pool("psum_o", 4, "PSUM")
    for tt in range(NQ):
        for nt in range(2):
            ps = pso.tile([128, 512], F32, tag="ps_o")
            for cm in range(NQ):
                nc.tensor.matmul(out=ps[:], lhsT=attnT[:, cm, tt * 128:(tt + 1) * 128],
                                 rhs=wo[:, cm, nt * 512:(nt + 1) * 512],
                                 start=(cm == 0), stop=(cm == NQ - 1))
            sl = slice(nt * 512, (nt + 1) * 512)
            nc.scalar.copy(out=dsb[:, tt, sl], in_=ps[:])
            # h2 = o_proj + tokens, in place over cb
            nc.vector.scalar_tensor_tensor(
                out=cb[:, tt, sl], in0=ps[:], scalar=1.0, in1=cb[:, tt, sl],
                op0=Alu.mult, op1=Alu.add)
            if nt == 1:
                nc.vector.tensor_reduce(out=ssum2[:, tt:tt + 1],
                                        in_=cb[:, tt, :],
                                        axis=mybir.AxisListType.X, op=Alu.add)
                nc.vector.scalar_tensor_tensor(
                    out=junk[:], in0=cb[:, tt, :], scalar=1.0,
                    in1=cb[:, tt, :], op0=Alu.mult, op1=Alu.mult,
                    accum_out=ssq2[:, tt:tt + 1])
    close(pso, att_p, wop, vbp, qkp)
    hsb = cb
    dump("hsb", hsb[:])
    if last_stage < 6:
        return

    # ---------------- S6: LN2 -> mT --------------------------------------
    mu2 = rpool.tile([128, NQ], F32, name="mu2")
    rstd2 = rpool.tile([128, NQ], F32, name="rstd2")
    ln_stats(ssum2, ssq2, mu2, rstd2)
    mtp = pool("mT", 1)
    mT = mtp.tile([128, NQ, K], FP8 if FP8_FC else BF16)
    mbfp = pool("mbf", 1)
    mbf = mbfp.tile([128, NQ, C], BF16)
    for kc in range(NQ):
        nc.vector.tensor_scalar(out=mbf[:, kc, :], in0=hsb[:, kc, :],
                                scalar1=mu2[:, kc:kc + 1], scalar2=rstd2[:, kc:kc + 1],
                                op0=Alu.subtract, op1=Alu.mult)
    ptp2 = pool("psum_t2", 4, "PSUM")
    transpose_block(mbf, mT, NQ, NQ, ptp2)
    close(ptp2, mbfp)
    if last_stage < 7:
        return

    # ---------------- S7: fc + gelu --------------------------------------
    gtp = pool("gT", 1, side="right")
    gT = gtp.tile([128, DFF // 128, K], FP8 if FP8_PROJ else BF16)
    wfp = pool("wfc", 3)
    psf = pool("psum_f", 4, "PSUM")
    NQT = DFF // 8
    DR = mybir.MatmulPerfMode.DoubleRow
    for qtr in range(8):
        if FP8_FC:
            wf = wfp.tile([128, C // 256, 2, NQT], FP8, tag="wf")
            nc.sync.dma_start(
                out=wf[:],
                in_=io["wfcT8"].ap()[:, :, :, qtr * NQT:(qtr + 1) * NQT]
                .rearrange("c i p f -> p c i f"))
        else:
            wf = wfp.tile([128, NQ, NQT], BF16, tag="wf")
            nc.sync.dma_start(
                out=wf[:],
                in_=io["wfcT"].ap()[:, qtr * NQT:(qtr + 1) * NQT]
                .rearrange("(cc p) f -> p cc f", p=128))
        for fm in range(NQT // 128):
            ffm = qtr * (NQT // 128) + fm
            for nt in range(2):
                ps = psf.tile([128, 512], F32, tag="ps_f")
                if FP8_FC:
                    for c2 in range(C // 256):
                        nc.tensor.matmul(out=ps[:],
                                         lhsT=wf[:, c2, :, fm * 128:(fm + 1) * 128],
                                         rhs=mT[:, 2 * c2:2 * c2 + 2,
                                                nt * 512:(nt + 1) * 512],
                                         start=(c2 == 0),
                                         stop=(c2 == C // 256 - 1), perf_mode=DR)
                    nc.scalar.activation(out=gT[:, ffm, nt * 512:(nt + 1) * 512],
                                         in_=ps[:], func=Act.Gelu,
                                         scale=1.0 / WSCALE)
                else:
                    for cc in range(NQ):
                        nc.tensor.matmul(out=ps[:], lhsT=wf[:, cc, fm * 128:(fm + 1) * 128],
                                         rhs=mT[:, cc, nt * 512:(nt + 1) * 512],
                                         start=(cc == 0), stop=(cc == NQ - 1))
                    nc.scalar.activation(out=gT[:, ffm, nt * 512:(nt + 1) * 512],
                                         in_=ps[:], func=Act.Gelu)
    close(psf, wfp, mtp, lnp, cbp)
    dump("gT", gT[:])
    if last_stage < 8:
        return

    # ---------------- S8: proj + delta = proj_out + dsb (in place) --------
    delta = dsb
    psp = pool("psum_p", 4, "PSUM")
    if FP8_PROJ:
        wpp = pool("wproj", 1)
        wp8 = wpp.tile([128, DFF // 256, 2, C], FP8)
        nc.sync.dma_start(out=wp8[:],
                          in_=io["wprojT8"].ap().rearrange("f i p c -> p f i c"))
        for tt in range(NQ):
            for nt in range(2):
                ps = psp.tile([128, 512], F32, tag="ps_p")
                for f2 in range(DFF // 256):
                    nc.tensor.matmul(out=ps[:],
                                     lhsT=gT[:, 2 * f2:2 * f2 + 2,
                                             tt * 128:(tt + 1) * 128],
                                     rhs=wp8[:, f2, :, nt * 512:(nt + 1) * 512],
                                     start=(f2 == 0), stop=(f2 == DFF // 256 - 1),
                                     perf_mode=DR)
                sl = slice(nt * 512, (nt + 1) * 512)
                nc.vector.scalar_tensor_tensor(out=delta[:, tt, sl], in0=ps[:],
                                               scalar=1.0 / WSCALE,
                                               in1=dsb[:, tt, sl],
                                               op0=Alu.mult, op1=Alu.add)
    else:
        wpp = pool("wproj", 1)
        wps = []
        for half in range(2):
            wp = wpp.tile([128, DFF // 256, C], BF16, name=f"wp{half}")
            nc.sync.dma_start(
                out=wp[:],
                in_=io["wprojT"].ap()[half * DFF // 2:(half + 1) * DFF // 2, :]
                .rearrange("(fc p) c -> p fc c", p=128))
            wps.append(wp)
        for tt in range(NQ):
            for nt in range(2):
                ps = psp.tile([128, 512], F32, tag="ps_p")
                for fc2 in range(DFF // 128):
                    wp = wps[fc2 // (DFF // 256)]
                    fm = fc2 % (DFF // 256)
                    nc.tensor.matmul(out=ps[:], lhsT=gT[:, fc2, tt * 128:(tt + 1) * 128],
                                     rhs=wp[:, fm, nt * 512:(nt + 1) * 512],
                                     start=(fc2 == 0), stop=(fc2 == DFF // 128 - 1))
                sl = slice(nt * 512, (nt + 1) * 512)
                nc.vector.scalar_tensor_tensor(out=delta[:, tt, sl], in0=ps[:],
                                               scalar=1.0, in1=dsb[:, tt, sl],
                                               op0=Alu.mult, op1=Alu.add)
    close(psp, wpp, gtp)
    dump("delta", delta[:])
    if last_stage < 9:
        return

    # ---------------- S9: scatter matmul + combine + store ----------------
    xqp = pool("xq", 3)
    fin = pool("fin", 2)
    ps_sc = pool("psum_sc", 2, "PSUM")
    outr = out.rearrange("(c p) d -> p c d", p=128)

    def scatter_tile(tc, xq):
        ps = ps_sc.tile([128, C], F32, tag="ps_sc")
        for nt in range(2):
            for kc in range(NQ):
                nc.tensor.matmul(out=ps[:, nt * 512:(nt + 1) * 512],
                                 lhsT=pw[:, kc, tc * 128:(tc + 1) * 128],
                                 rhs=delta[:, kc, nt * 512:(nt + 1) * 512],
                                 start=(kc == 0), stop=(kc == NQ - 1))
        res = fin.tile([128, C], F32, tag="res")
        for nt in range(2):
            sl = slice(nt * 512, (nt + 1) * 512)
            nc.vector.scalar_tensor_tensor(out=res[:, sl], in0=xq[:, sl],
                                           scalar=wm1[:, tc:tc + 1], in1=ps[:, sl],
                                           op0=Alu.mult, op1=Alu.add)
        nc.sync.dma_start(out=outr[:, tc, :], in_=res[:])

    xqs = []
    for tc in range(NCH):
        xq = xqp.tile([128, C], F32, tag="xq")
        nc.sync.dma_start(out=xq[:], in_=xbr[:, tc, :])
        xqs.append(xq)
        if tc >= 2:
            scatter_tile(tc - 2, xqs[tc - 2])
    scatter_tile(NCH - 2, xqs[NCH - 2])
    scatter_tile(NCH - 1, xqs[NCH - 1])
    close(ps_sc, fin, xqp, dsp, pwp)


_CACHED = {}


def _get_program(dbg_names=(), last_stage=99):
    key = (tuple(dbg_names), last_stage)
    if key not in _CACHED:
        nc = bass.Bass("TRN2", target_bir_lowering=False, debug=False)
        io, dbg = declare_io(nc, dbg_names)
        with FunnelTileContext(nc) as tc:
            build(nc, tc, io, dbg, last_stage=last_stage)
        fix_sync_waits(nc)
        _CACHED[key] = nc
    return _CACHED[key]


def kernel(**inputs) -> np.ndarray:
    nc = _get_program()
    in_maps = host_inputs(inputs)
    res = run_bass_kernel_spmd(nc, in_maps, core_ids=list(range(B)))
    return np.stack([np.asarray(res.results[b]["out"], np.float32)
                     for b in range(B)])


# revision 5
# speedup vs baseline: 1.0047x; 1.0047x over previous
"""Mixture-of-Depths block kernel v2 for 8 TRN2 NeuronCores (Bass/Tile).

Data-parallel over batch B=8, one batch row per core. v2 replaces v1's
indirect-DMA gather/scatter through a DRAM compact buffer with one-hot
matmul gather/scatter on the tensor engine:

  S0  stream x in 128-token chunks; exact-fp32 router logits (DVE+Pool),
      16-ary threshold search, prefix-sum slot positions o_f (>=K for
      unselected tokens), router weights w_tok, and a DRAM staging row
      of (o_f, w_tok) for the scatter one-hot.
  S1  one-hot P[t,k]=(o_f[t]==k) via is_equal; x cast to bf16; gather
      matmul cb[k,:] = P^T x (eviction fused with LN1 row-sum); scatter
      one-hot Pw[k,t]=w[t]*(o_f[t]==k) built from the broadcast staging
      row (fills DVE/Pool slack under the gather matmul).
  S2  LN1 (fused sumsq, stats on [128,8] vectors) -> anorm -> aT.
  S3  QKV.  S4 causal attention (ones-row V trick for denominators).
  S5  o_proj WITHOUT residual (dsb) + h2 = dsb + cb in-place (fused
      LN2 row-sum).  S6 LN2 -> mT.  S7 fc + gelu -> gT.  S8 proj +
      delta = proj_out + dsb.  S9 scatter matmul out_tok = Pw^T delta,
      combined as out = x*(1 + w*sel) + scatter, streamed per t-chunk.

out[t] = x[t]*(1+w[t]*sel[t]) + sum_k Pw[k,t]*(attn_out+mlp)[k] equals
the reference's x.at[idx].add(w * processed) because processed =
tokens + attn_out + mlp and the tokens term collapses to w[t]*x[t].
"""
import numpy as np
import ml_dtypes

import concourse.bass as bass
import concourse.mybir as mybir
import concourse.tile as tile
from concourse.bass_utils import run_bass_kernel_spmd
from concourse.vector_clock import ScopedClock, VectorClock

dt = mybir.dt
Alu = mybir.AluOpType
Act = mybir.ActivationFunctionType

MAX_WAITS = 1


def fix_sync_waits(nc, max_waits=MAX_WAITS):
    n_split = 0
    for f in nc.m.functions:
        for bb in f.blocks:
            new = []
            for inst in bb.instructions:
                si = inst.sync_info
                if si is not None and si.on_wait and len(si.on_wait) > max_waits:
                    waits = list(si.on_wait)
                    extra, keep = waits[:-max_waits], waits[-max_waits:]
                    for w in extra:
                        n_split += 1
                        nop = mybir.InstNoOp(name=f"{inst.name}-ws{n_split}")
                        nop.engine = inst.engine
                        nop.sync_info = mybir.SyncInfo(on_wait=[w], on_update=[])
                        new.append(nop)
                    inst.sync_info = mybir.SyncInfo(
                        on_wait=keep, on_update=list(si.on_update))
                new.append(inst)
            bb.instructions[:] = new
    return n_split


class FunnelTileContext(tile.TileContext):
    """TileContext whose tail drain's waits are split across funnel drains."""

    def _drain_and_barrier(self, tick_clock, wait_clock):
        gc = tick_clock.global_clock
        ticks = eval(repr(gc).replace('VectorClock(', '').rstrip(')'))
        for i, t in enumerate(ticks):
            if t > 0:
                partial = [0] * 27
                partial[i] = t
                d = self.nc.sync.drain()
                wait_clock.add_sem_waits(d.ins, ScopedClock({None: VectorClock(partial)}))
        self.nc.sync.drain()
        self.nc.all_engine_barrier()
        assert self.sems is not None
        popped = self.nc._tile_sem_poison_stack.pop()
        assert popped is self._sem_poison
        sems = list(self.sems.allocated().values())
        for i in range(0, len(sems), 8):
            self.nc.clear_and_free_semaphores(sems[i:i + 8])
        self.nc.all_engine_barrier()


B, T, C = 8, 2048, 1024
K = 1024
H = 16
DH = C // H
DFF = 4 * C
EPS = 1e-5
NCH = T // 128    # 16
NQ = K // 128     # 8
SRCH_ITERS = 7
LO0, STEP0 = -8.0, 1.0

F32, BF16, I32, I16 = dt.float32, dt.bfloat16, dt.int32, dt.int16
FP8 = dt.float8e4
FP8_QKV = True      # fp8 DoubleRow QKV projection (error-budget tested)
FP8_FC = False      # fc/proj stay bf16: fp8 there breaks the 2e-2 budget
FP8_PROJ = False
WSCALE = 16.0       # fp8 weight prescale (descaled at PSUM eviction)


def host_inputs(inputs):
    x = np.asarray(inputs["x"], np.float32)
    assert x.shape == (B, T, C)
    assert int(inputs["top_k"]) == K and int(inputs["n_head"]) == H

    def bf(a):
        return np.ascontiguousarray(np.asarray(a, np.float32)).astype(ml_dtypes.bfloat16)

    def f8(aT, pairs):
        # [R, F] -> [R/256, 2, 128, F] fp8 with rows prescaled by WSCALE
        a = np.ascontiguousarray(np.asarray(aT, np.float32)) * WSCALE
        R, F = a.shape
        return a.reshape(R // 256, 2, 128, F).astype(ml_dtypes.float8_e4m3fn)

    common = {
        "wrt128": np.ascontiguousarray(np.broadcast_to(
            np.asarray(inputs["w_router"], np.float32), (128, C))),
        "woT": bf(np.asarray(inputs["w_o"], np.float32).T),
        "stair": bf(np.triu(np.ones((128, 128), np.float32))),
        "iota15": np.ascontiguousarray(np.broadcast_to(
            np.arange(1, 16, dtype=np.float32), (128, 15))),
        "iotaT": np.ascontiguousarray(
            np.arange(T, dtype=np.float32).reshape(NCH, 128).T),
        "utri": np.triu(np.ones((128, 128), np.float32), 1),
        "ones2d": np.ones((128, 128), np.float32),
        "onesbf": bf(np.ones((128, 128), np.float32)),
        "ident_bf": bf(np.eye(128, dtype=np.float32)),
        "iotaK": np.ascontiguousarray(np.broadcast_to(
            np.arange(K, dtype=np.int16), (128, K))),
    }
    if FP8_QKV:
        common["wqkvT8"] = f8(np.asarray(inputs["w_qkv"], np.float32).T, 2)
    else:
        common["wqkvT"] = bf(np.asarray(inputs["w_qkv"], np.float32).T)
    if FP8_FC:
        common["wfcT8"] = f8(np.asarray(inputs["w_fc"], np.float32).T, 2)
    else:
        common["wfcT"] = bf(np.asarray(inputs["w_fc"], np.float32).T)
    if FP8_PROJ:
        common["wprojT8"] = f8(np.asarray(inputs["w_proj"], np.float32).T, 2)
    else:
        common["wprojT"] = bf(np.asarray(inputs["w_proj"], np.float32).T)
    for nm in ("ln1_w", "ln2_w"):
        assert np.all(np.asarray(inputs[nm]) == 1), nm
    for nm in ("ln1_b", "ln2_b", "b_qkv", "b_o", "b_fc", "b_proj"):
        assert np.all(np.asarray(inputs[nm]) == 0), nm

    return [dict(common, xb=np.ascontiguousarray(x[b])) for b in range(B)]


def declare_io(nc, dbg_names=()):
    io = {}
    io["xb"] = nc.dram_tensor("xb", [T, C], F32, kind="ExternalInput")
    io["wrt128"] = nc.dram_tensor("wrt128", [128, C], F32, kind="ExternalInput")
    io["woT"] = nc.dram_tensor("woT", [C, C], BF16, kind="ExternalInput")
    if FP8_QKV:
        io["wqkvT8"] = nc.dram_tensor("wqkvT8", [C // 256, 2, 128, 3 * C], FP8,
                                      kind="ExternalInput")
    else:
        io["wqkvT"] = nc.dram_tensor("wqkvT", [C, 3 * C], BF16, kind="ExternalInput")
    if FP8_FC:
        io["wfcT8"] = nc.dram_tensor("wfcT8", [C // 256, 2, 128, DFF], FP8,
                                     kind="ExternalInput")
    else:
        io["wfcT"] = nc.dram_tensor("wfcT", [C, DFF], BF16, kind="ExternalInput")
    if FP8_PROJ:
        io["wprojT8"] = nc.dram_tensor("wprojT8", [DFF // 256, 2, 128, C], FP8,
                                       kind="ExternalInput")
    else:
        io["wprojT"] = nc.dram_tensor("wprojT", [DFF, C], BF16, kind="ExternalInput")
    io["stair"] = nc.dram_tensor("stair", [128, 128], BF16, kind="ExternalInput")
    io["iota15"] = nc.dram_tensor("iota15", [128, 15], F32, kind="ExternalInput")
    io["iotaT"] = nc.dram_tensor("iotaT", [128, NCH], F32, kind="ExternalInput")
    io["utri"] = nc.dram_tensor("utri", [128, 128], F32, kind="ExternalInput")
    io["ones2d"] = nc.dram_tensor("ones2d", [128, 128], F32, kind="ExternalInput")
    io["onesbf"] = nc.dram_tensor("onesbf", [128, 128], BF16, kind="ExternalInput")
    io["ident_bf"] = nc.dram_tensor("ident_bf", [128, 128], BF16, kind="ExternalInput")
    io["iotaK"] = nc.dram_tensor("iotaK", [128, K], I16, kind="ExternalInput")
    io["out"] = nc.dram_tensor("out", [T, C], F32, kind="ExternalOutput")
    io["rowscr"] = nc.dram_tensor("rowscr", [2, T], F32, kind="Internal")
    dbg = {}
    shapes = {"o_f": ([128, NCH], F32), "ls": ([128, NCH], F32),
              "lo": ([128, 1], F32), "cb": ([128, NQ, C], BF16),
              "anorm": ([128, NQ, C], BF16), "aT": ([128, NQ, K], BF16),
              "qk": ([128, 2 * NQ, K], BF16), "attnT": ([128, NQ, K], BF16),
              "hsb": ([128, NQ, C], BF16), "gT": ([128, DFF // 128, K], BF16),
              "delta": ([128, NQ, C], BF16), "pw": ([128, NQ, T], BF16)}
    for nm in dbg_names:
        sh, d = shapes[nm]
        dbg[nm] = nc.dram_tensor("dbg_" + nm, sh, d, kind="ExternalOutput")
    return io, dbg


def build(nc, tc, io, dbg=None, last_stage=99):
    opened = []
    try:
        _build(nc, tc, io, dbg or {}, last_stage, opened)
    finally:
        for p in reversed(opened):
            p._cm.__exit__(None, None, None)


def _build(nc, tc, io, dbg, last_stage, opened):
    def pool(name, bufs, space=None, side="left"):
        kw = {"space": space} if space else {}
        if not space:
            kw["side"] = side
        cm = tc.tile_pool(name=name, bufs=bufs, **kw)
        p = cm.__enter__()
        p._cm = cm
        p._side = kw.get("side", "psum")
        opened.append(p)
        return p

    def close(*ps):
        for p in sorted(ps, key=opened.index, reverse=True):
            same = [q for q in opened if q._side == p._side]
            assert same[-1] is p, (p.name, [q.name for q in opened])
            opened.remove(p)
            p._cm.__exit__(None, None, None)

    xb, out = io["xb"].ap(), io["out"].ap()
    rowscr = io["rowscr"].ap()
    xbr = xb.rearrange("(c p) d -> p c d", p=128)

    def dump(nm, ap_or_tile):
        if nm in dbg:
            nc.sync.dma_start(out=dbg[nm].ap(), in_=ap_or_tile)

    cpool = pool("const", 1)
    consts = {}
    for nm, shape, d in (("wrt128", [128, C], F32), ("stair", [128, 128], BF16),
                         ("iota15", [128, 15], F32), ("iotaT", [128, NCH], F32),
                         ("utri", [128, 128], F32), ("ones2d", [128, 128], F32),
                         ("onesbf", [128, 128], BF16), ("ident_bf", [128, 128], BF16),
                         ("iotaK", [128, K], I16)):
        t = cpool.tile(shape, d, name="c_" + nm)
        nc.sync.dma_start(out=t[:], in_=io[nm].ap())
        consts[nm] = t
    wrt, stair, iota15, iotaT = (consts["wrt128"], consts["stair"],
                                 consts["iota15"], consts["iotaT"])
    utri, ones2d, onesbf, ident = (consts["utri"], consts["ones2d"],
                                   consts["onesbf"], consts["ident_bf"])
    iotaK = consts["iotaK"]

    # long-lived small state
    rpool = pool("router", 1)
    epsc = rpool.tile([128, 1], F32)
    nc.vector.memset(epsc[:], EPS)
    pofs_i = rpool.tile([128, NQ], I32)
    nc.gpsimd.iota(pofs_i[:], pattern=[[128, NQ]], base=0, channel_multiplier=1)
    pofs = rpool.tile([128, NQ], F32)
    nc.gpsimd.tensor_copy(pofs[:], pofs_i[:])
    junk = rpool.tile([128, C], F32, name="junk")

    # cb: gathered tokens, then h2 in-place; lives S1..end of fc phase
    cbp = pool("cb", 1)
    cb = cbp.tile([128, NQ, C], BF16)
    stats = rpool.tile([128, NQ], F32, name="ssum8")
    ssq8 = rpool.tile([128, NQ], F32, name="ssq8")

    # ---------------- S0: stream x chunks, router, top-k ------------------
    ppool = pool("ponehot", 1)
    P = ppool.tile([128, NCH, K], BF16)
    xbfp = pool("xbf", 1)
    xbf = xbfp.tile([128, NCH, C], BF16)
    xsp = pool("xs", 6)
    ls = rpool.tile([128, NCH], F32)
    for c in range(NCH):
        xsc = xsp.tile([128, C], F32, tag="xsc")
        nc.sync.dma_start(out=xsc[:], in_=xbr[:, c, :])
        # exact-fp32 router logits (fused multiply+reduce on DVE)
        nc.vector.scalar_tensor_tensor(
            out=junk[:], in0=xsc[:], scalar=1.0, in1=wrt[:],
            op0=Alu.mult, op1=Alu.mult, accum_out=ls[:, c:c + 1])
        # bf16 cast for the gather matmul (Activation engine, idle here)
        nc.scalar.copy(out=xbf[:, c, :], in_=xsc[:])
    close(xsp)

    lo = rpool.tile([128, 1], F32)
    step = rpool.tile([128, 1], F32)
    nc.vector.memset(lo[:], LO0)
    nc.vector.memset(step[:], STEP0)
    mids = rpool.tile([128, 15], F32)
    cmp3 = rpool.tile([128, 15, NCH], F32)
    red = rpool.tile([128, 15], F32)
    scrap = rpool.tile([128, 15], F32)
    nbuk = rpool.tile([128, 1], F32)
    psum_srch = pool("psum_srch", 2, "PSUM")
    for it in range(SRCH_ITERS):
        nc.vector.scalar_tensor_tensor(
            out=mids[:], in0=iota15[:], scalar=step[:, 0:1],
            in1=lo[:, 0:1].to_broadcast([128, 15]), op0=Alu.mult, op1=Alu.add)
        nc.vector.tensor_tensor(
            out=cmp3[:], in0=ls[:].unsqueeze(1).to_broadcast([128, 15, NCH]),
            in1=mids[:].unsqueeze(2).to_broadcast([128, 15, NCH]), op=Alu.is_gt)
        nc.vector.tensor_reduce(out=red[:], in_=cmp3[:], axis=mybir.AxisListType.X,
                                op=Alu.add)
        cnt = psum_srch.tile([128, 15], F32, tag="cnt")
        nc.tensor.matmul(out=cnt[:], lhsT=ones2d[:], rhs=red[:], start=True, stop=True)
        nc.vector.tensor_scalar(out=scrap[:], in0=cnt[:], scalar1=float(K),
                                scalar2=None, op0=Alu.is_ge, op1=Alu.add,
                                accum_out=nbuk[:])
        nc.vector.scalar_tensor_tensor(out=lo[:], in0=nbuk[:], scalar=step[:, 0:1],
                                       in1=lo[:], op0=Alu.mult, op1=Alu.add)
        nc.vector.tensor_scalar_mul(step[:], step[:], 1.0 / 16.0)

    mask = rpool.tile([128, NCH], F32)
    nc.vector.tensor_scalar(out=mask[:], in0=ls[:], scalar1=lo[:, 0:1],
                            scalar2=None, op0=Alu.is_gt)
    pre = psum_srch.tile([128, NCH], F32, tag="pre")
    nc.tensor.matmul(out=pre[:], lhsT=utri[:], rhs=mask[:], start=True, stop=True)
    tot = psum_srch.tile([128, NCH], F32, tag="tot")
    nc.tensor.matmul(out=tot[:], lhsT=ones2d[:], rhs=mask[:], start=True, stop=True)
    ex = rpool.tile([128, NCH], F32)
    ex2 = rpool.tile([128, NCH], F32)
    nc.vector.memset(ex[:, 0:1], 0.0)
    nc.vector.tensor_copy(ex[:, 1:NCH], tot[:, 0:NCH - 1])
    cur, nxt = ex, ex2
    for d in (1, 2, 4, 8):
        nc.vector.tensor_copy(nxt[:, 0:d], cur[:, 0:d])
        nc.vector.tensor_tensor(out=nxt[:, d:NCH], in0=cur[:, d:NCH],
                                in1=cur[:, 0:NCH - d], op=Alu.add)
        cur, nxt = nxt, cur
    pos = rpool.tile([128, NCH], F32)
    nc.vector.tensor_tensor(out=pos[:], in0=pre[:], in1=cur[:], op=Alu.add)
    alt = rpool.tile([128, NCH], F32)
    nc.vector.scalar_tensor_tensor(out=alt[:], in0=iotaT[:], scalar=float(K),
                                   in1=pos[:], op0=Alu.add, op1=Alu.subtract)
    dif = rpool.tile([128, NCH], F32)
    nc.vector.tensor_tensor(out=dif[:], in0=pos[:], in1=alt[:], op=Alu.subtract)
    nc.vector.tensor_tensor(out=dif[:], in0=dif[:], in1=mask[:], op=Alu.mult)
    o_f = rpool.tile([128, NCH], F32)
    nc.vector.tensor_tensor(out=o_f[:], in0=alt[:], in1=dif[:], op=Alu.add)
    w_tok = rpool.tile([128, NCH], F32)
    nc.vector.tensor_tensor(out=w_tok[:], in0=ls[:], in1=mask[:], op=Alu.mult)
    wm1 = rpool.tile([128, NCH], F32)
    nc.vector.tensor_scalar(out=wm1[:], in0=w_tok[:], scalar1=1.0,
                            scalar2=None, op0=Alu.add)
    nc.sync.dma_start(out=rowscr[0, :].rearrange("(c p) -> p c", p=128),
                      in_=o_f[:])
    nc.sync.dma_start(out=rowscr[1, :].rearrange("(c p) -> p c", p=128),
                      in_=w_tok[:])
    close(psum_srch)

    dump("o_f", o_f[:])
    dump("ls", ls[:])
    dump("lo", lo[:])
    if last_stage < 1:
        return

    # ---------------- S1: gather one-hot + gather matmul ------------------
    # right-side pools: pw (scatter one-hot, lives to S9), wq (QKV weights,
    # prefetched now), bcb (broadcast o_f/w rows, transient)
    pwp = pool("pw", 1, side="right")
    pw = pwp.tile([128, NQ, T], BF16)
    if FP8_QKV:
        wqp = pool("wqkv", 1, side="right")
        wq = wqp.tile([128, C // 256, 2, 3 * C], FP8)
        nc.sync.dma_start(out=wq[:],
                          in_=io["wqkvT8"].ap().rearrange("c i p f -> p c i f"))
    else:
        wqp = pool("wqkv", 1, side="right")
        wq = wqp.tile([128, NQ, 3 * C], BF16)
        nc.sync.dma_start(out=wq[:],
                          in_=io["wqkvT"].ap().rearrange("(cc p) f -> p cc f", p=128))
    bcbp = pool("bcb", 1, side="right")
    bcb = bcbp.tile([128, 2, T], F32)
    nc.sync.dma_start(out=bcb[:],
                      in_=rowscr.unsqueeze(0).to_broadcast([128, 2, T]))

    for c in range(NCH):
        nc.vector.tensor_scalar(out=P[:, c, :], in0=iotaK[:],
                                scalar1=o_f[:, c:c + 1],
                                scalar2=None, op0=Alu.is_equal)

    psum_g = pool("psum_g", 2, "PSUM")
    for kc in range(NQ):
        pg = psum_g.tile([128, C], F32, tag="pg")
        for tc2 in range(2):
            for c in range(NCH):
                nc.tensor.matmul(out=pg[:, tc2 * 512:(tc2 + 1) * 512],
                                 lhsT=P[:, c, kc * 128:(kc + 1) * 128],
                                 rhs=xbf[:, c, tc2 * 512:(tc2 + 1) * 512],
                                 start=(c == 0), stop=(c == NCH - 1))
        nc.vector.tensor_copy(cb[:, kc, :], pg[:])
        nc.vector.tensor_reduce(out=stats[:, kc:kc + 1], in_=pg[:],
                                axis=mybir.AxisListType.X, op=Alu.add)
        nc.vector.scalar_tensor_tensor(
            out=junk[:], in0=cb[:, kc, :], scalar=1.0, in1=cb[:, kc, :],
            op0=Alu.mult, op1=Alu.mult, accum_out=ssq8[:, kc:kc + 1])
        # scatter one-hot build rides the DVE slack under the gather matmul
        nc.vector.scalar_tensor_tensor(out=pw[:, kc, :], in0=bcb[:, 0, :],
                                       scalar=pofs[:, kc:kc + 1], in1=bcb[:, 1, :],
                                       op0=Alu.is_equal, op1=Alu.mult)
    close(psum_g, xbfp, ppool, bcbp)
    dump("pw", pw[:])
    dump("cb", cb[:])
    if last_stage < 2:
        return

    # ---------------- S2: LN1 + transpose to aT ---------------------------
    lnp = pool("ln", 2)

    def ln_stats(ssum8, sq8, mu8, rstd8):
        nc.vector.tensor_scalar_mul(mu8[:], ssum8[:], 1.0 / C)
        nmu2 = lnp.tile([128, NQ], F32, tag="nmu2")
        nc.vector.tensor_tensor(out=nmu2[:], in0=mu8[:], in1=mu8[:], op=Alu.mult)
        var8 = lnp.tile([128, NQ], F32, tag="var8")
        nc.vector.scalar_tensor_tensor(out=var8[:], in0=sq8[:], scalar=1.0 / C,
                                       in1=nmu2[:], op0=Alu.mult, op1=Alu.subtract)
        lgv = lnp.tile([128, NQ], F32, tag="lgv")
        nc.scalar.activation(out=lgv[:], in_=var8[:], func=Act.Ln, bias=epsc[:, 0:1])
        nc.scalar.activation(out=rstd8[:], in_=lgv[:], func=Act.Exp, scale=-0.5)

    mu8 = rpool.tile([128, NQ], F32, name="mu8")
    rstd8 = rpool.tile([128, NQ], F32, name="rstd8")
    ln_stats(stats, ssq8, mu8, rstd8)

    qkp = pool("qk", 1)
    qk = qkp.tile([128, 2 * NQ, K], BF16)
    vbp = pool("vb", 1)
    vb = vbp.tile([128, NQ, H * (DH + 1)], BF16)
    atp = pool("aT", 1)
    aT = atp.tile([128, NQ, K], FP8 if FP8_QKV else BF16)
    anp = pool("anorm", 1)
    anorm = anp.tile([128, NQ, C], BF16)
    for kc in range(NQ):
        nc.vector.tensor_scalar(out=anorm[:, kc, :], in0=cb[:, kc, :],
                                scalar1=mu8[:, kc:kc + 1], scalar2=rstd8[:, kc:kc + 1],
                                op0=Alu.subtract, op1=Alu.mult)
    dump("anorm", anorm[:])

    def transpose_block(src3, dst3, n_row, n_col, tp):
        # j2 outer so low c-tiles complete first (consumers read c-pairs
        # across all k); evictions alternate DVE/Act to halve the chain.
        for j2 in range(0, n_col, 4):
            jm = min(j2 + 4, n_col)
            for i in range(n_row):
                pt = tp.tile([128, 512], BF16, tag="pt")
                for j in range(j2, jm):
                    nc.tensor.transpose(out=pt[:, (j - j2) * 128:(j - j2 + 1) * 128],
                                        in_=src3[:, i, j * 128:(j + 1) * 128],
                                        identity=ident[:])
                dst = dst3[:, j2:jm, i * 128:(i + 1) * 128]
                src = pt[:, 0:(jm - j2) * 128].rearrange("p (j d) -> p j d", d=128)
                if i % 2 == 0:
                    nc.scalar.copy(out=dst, in_=src)
                else:
                    nc.vector.tensor_copy(dst, src)

    ptp1 = pool("psum_t1", 4, "PSUM")
    transpose_block(anorm, aT, NQ, NQ, ptp1)
    close(ptp1, anp)
    dump("aT", aT[:])
    if last_stage < 3:
        return

    # ---------------- S3: QKV -------------------------------------------
    if FP8_QKV:
        pqk = pool("psum_qk", 4, "PSUM")
        DR = mybir.MatmulPerfMode.DoubleRow
        for mf in range(2 * NQ):
            for nt in range(2):
                ps = pqk.tile([128, 512], F32, tag="ps")
                for c2 in range(C // 256):
                    nc.tensor.matmul(out=ps[:],
                                     lhsT=wq[:, c2, :, mf * 128:(mf + 1) * 128],
                                     rhs=aT[:, 2 * c2:2 * c2 + 2,
                                            nt * 512:(nt + 1) * 512],
                                     start=(c2 == 0), stop=(c2 == C // 256 - 1),
                                     perf_mode=DR)
                nc.vector.tensor_scalar(out=qk[:, mf, nt * 512:(nt + 1) * 512],
                                        in0=ps[:], scalar1=1.0 / WSCALE,
                                        scalar2=None, op0=Alu.mult)
        for tt in range(NQ):
            for nt in range(2):
                ps = pqk.tile([128, 512], F32, tag="ps")
                for c2 in range(C // 256):
                    nc.tensor.matmul(
                        out=ps[:],
                        lhsT=aT[:, 2 * c2:2 * c2 + 2, tt * 128:(tt + 1) * 128],
                        rhs=wq[:, c2, :, 2 * C + nt * 512:2 * C + (nt + 1) * 512],
                        start=(c2 == 0), stop=(c2 == C // 256 - 1), perf_mode=DR)
                dst = vb[:, tt, :].rearrange("p (h d) -> p h d", d=DH + 1)
                nc.vector.tensor_scalar(
                    out=dst[:, nt * 8:(nt + 1) * 8, 0:DH],
                    in0=ps[:].rearrange("p (h d) -> p h d", d=DH),
                    scalar1=1.0 / WSCALE, scalar2=None, op0=Alu.mult)
    else:
        pqk = pool("psum_qk", 4, "PSUM")
        for mf in range(2 * NQ):
            for nt in range(2):
                ps = pqk.tile([128, 512], F32, tag="ps")
                for cc in range(NQ):
                    nc.tensor.matmul(out=ps[:], lhsT=wq[:, cc, mf * 128:(mf + 1) * 128],
                                     rhs=aT[:, cc, nt * 512:(nt + 1) * 512],
                                     start=(cc == 0), stop=(cc == NQ - 1))
                nc.vector.tensor_copy(qk[:, mf, nt * 512:(nt + 1) * 512], ps[:])
        for tt in range(NQ):
            for nt in range(2):
                ps = pqk.tile([128, 512], F32, tag="ps")
                for cc in range(NQ):
                    nc.tensor.matmul(out=ps[:], lhsT=aT[:, cc, tt * 128:(tt + 1) * 128],
                                     rhs=wq[:, cc, 2 * C + nt * 512:2 * C + (nt + 1) * 512],
                                     start=(cc == 0), stop=(cc == NQ - 1))
                dst = vb[:, tt, :].rearrange("p (h d) -> p h d", d=DH + 1)
                nc.vector.tensor_copy(dst[:, nt * 8:(nt + 1) * 8, 0:DH],
                                      ps[:].rearrange("p (h d) -> p h d", d=DH))
    ones_col = vb[:].rearrange("p q (h d) -> p q h d", d=DH + 1)[:, :, :, DH:DH + 1]
    nc.vector.memset(ones_col, 1.0)
    close(pqk, wqp, atp)
    dump("qk", qk[:])
    if last_stage < 4:
        return

    # ---------------- S4: attention --------------------------------------
    ssum2 = rpool.tile([128, NQ], F32, name="ssum2")
    ssq2 = rpool.tile([128, NQ], F32, name="ssq2")
    wop = pool("wo", 1)
    wo = wop.tile([128, NQ, C], BF16)
    nc.sync.dma_start(out=wo[:], in_=io["woT"].ap().rearrange("(cc p) f -> p cc f", p=128))
    att_p = pool("attnT", 1)
    attnT = att_p.tile([128, NQ, K], BF16)
    den_p = pool("den", 1)
    den_sb = den_p.tile([128, NQ, K], BF16)
    nump = pool("num", 1)
    ps_s = pool("psum_s", 2, "PSUM")
    ps_a = pool("psum_a", 2, "PSUM")
    rowp = pool("denrow", 2)

    for j in range(H // 2):
        nums = []
        for hh in range(2):
            h = 2 * j + hh
            p0 = 64 * hh
            num = nump.tile([128, NQ, K], BF16, tag=f"num{hh}")
            nums.append(num)
            mfK = NQ + j
            for kc in range(NQ):
                qlo = kc * 128
                ps = ps_s.tile([128, 1024], F32, tag="ps_s")
                for q0, q1 in ((qlo, min(qlo + 512, K)), (qlo + 512, K)):
                    if q1 <= q0:
                        continue
                    nc.tensor.matmul(
                        out=ps[:, q0 - qlo:q1 - qlo],
                        lhsT=qk[p0:p0 + DH, mfK, kc * 128:(kc + 1) * 128],
                        rhs=qk[p0:p0 + DH, j, q0:q1],
                        start=True, stop=True)
                nc.scalar.activation(out=num[:, kc, qlo:K],
                                     in_=ps[:, 0:K - qlo], func=Act.Exp,
                                     scale=0.125)
                dg = kc * 128
                nc.vector.tensor_tensor(out=num[:, kc, dg:dg + 128],
                                        in0=num[:, kc, dg:dg + 128],
                                        in1=stair[:], op=Alu.mult)
        for hh in range(2):
            h = 2 * j + hh
            num = nums[hh]
            for nt in range(2):
                pa = ps_a.tile([128, 512], F32, tag="ps_a")
                for qt in range(4 * nt, 4 * nt + 4):
                    co = qt * 128 - nt * 512
                    for kc in range(qt + 1):
                        nc.tensor.matmul(
                            out=pa[0:DH + 1, co:co + 128],
                            lhsT=vb[:, kc, h * (DH + 1):(h + 1) * (DH + 1)],
                            rhs=num[:, kc, qt * 128:(qt + 1) * 128],
                            start=(kc == 0), stop=(kc == qt))
                nc.vector.tensor_copy(
                    attnT[64 * hh:64 * hh + 64, j, nt * 512:(nt + 1) * 512],
                    pa[0:DH, :])
                drow = rowp.tile([128, 512], BF16, tag="drow")
                nc.vector.tensor_copy(drow[64:65, :], pa[DH:DH + 1, :])
                pd = ps_a.tile([128, 512], F32, tag="pd")
                nc.tensor.matmul(out=pd[0:64, :], lhsT=onesbf[64:65, 0:64],
                                 rhs=drow[64:65, :], start=True, stop=True)
                nc.vector.tensor_copy(
                    den_sb[64 * hh:64 * hh + 64, j, nt * 512:(nt + 1) * 512],
                    pd[0:64, :])
    close(rowp, ps_a, ps_s, nump)
    recp = pool("rec", 2)
    for cm in range(NQ):
        for nh in range(2):
            hsl = slice(nh * 512, (nh + 1) * 512)
            if cm % 2 == 0:
                # DVE reciprocal path keeps the Act queue clear for o_proj
                rcf = recp.tile([128, 512], F32, tag="lgd")
                nc.vector.reciprocal(rcf[:], den_sb[:, cm, hsl])
                nc.vector.tensor_tensor(out=attnT[:, cm, hsl],
                                        in0=attnT[:, cm, hsl],
                                        in1=rcf[:], op=Alu.mult)
            else:
                lgd = recp.tile([128, 512], F32, tag="lgd")
                nc.scalar.activation(out=lgd[:], in_=den_sb[:, cm, hsl], func=Act.Ln)
                rec = recp.tile([128, 512], BF16, tag="rec")
                nc.scalar.activation(out=rec[:], in_=lgd[:], func=Act.Exp, scale=-1.0)
                nc.vector.tensor_tensor(out=attnT[:, cm, hsl],
                                        in0=attnT[:, cm, hsl],
                                        in1=rec[:], op=Alu.mult)
    close(recp, den_p)
    dump("attnT", attnT[:])
    if last_stage < 5:
        return

    # ---------------- S5: o_proj (no residual) + h2 in-place --------------
    dsp = pool("dsb", 1, side="right")
    dsb = dsp.tile([128, NQ, C], BF16)
    pso = pool("psum_o", 4, "PSUM")
    for tt in range(NQ):
        for nt in range(2):
            ps = pso.tile([128, 512], F32, tag="ps_o")
            for cm in range(NQ):
                nc.tensor.matmul(out=ps[:], lhsT=attnT[:, cm, tt * 128:(tt + 1) * 128],
                                 rhs=wo[:, cm, nt * 512:(nt + 1) * 512],
                                 start=(cm == 0), stop=(cm == NQ - 1))
            sl = slice(nt * 512, (nt + 1) * 512)
            nc.scalar.copy(out=dsb[:, tt, sl], in_=ps[:])
            # h2 = o_proj + tokens, in place over cb
            nc.vector.scalar_tensor_tensor(
                out=cb[:, tt, sl], in0=ps[:], scalar=1.0, in1=cb[:, tt, sl],
                op0=Alu.mult, op1=Alu.add)
            if nt == 1:
                nc.vector.tensor_reduce(out=ssum2[:, tt:tt + 1],
                                        in_=cb[:, tt, :],
                                        axis=mybir.AxisListType.X, op=Alu.add)
                nc.vector.scalar_tensor_tensor(
                    out=junk[:], in0=cb[:, tt, :], scalar=1.0,
                    in1=cb[:, tt, :], op0=Alu.mult, op1=Alu.mult,
                    accum_out=ssq2[:, tt:tt + 1])
    close(pso, att_p, wop, vbp, qkp)
    hsb = cb
    dump("hsb", hsb[:])
    if last_stage < 6:
        return

    # ---------------- S6: LN2 -> mT --------------------------------------
    mu2 = rpool.tile([128, NQ], F32, name="mu2")
    rstd2 = rpool.tile([128, NQ], F32, name="rstd2")
    ln_stats(ssum2, ssq2, mu2, rstd2)
    mtp = pool("mT", 1)
    mT = mtp.tile([128, NQ, K], FP8 if FP8_FC else BF16)
    mbfp = pool("mbf", 1)
    mbf = mbfp.tile([128, NQ, C], BF16)
    for kc in range(NQ):
        nc.vector.tensor_scalar(out=mbf[:, kc, :], in0=hsb[:, kc, :],
                                scalar1=mu2[:, kc:kc + 1], scalar2=rstd2[:, kc:kc + 1],
                                op0=Alu.subtract, op1=Alu.mult)
    ptp2 = pool("psum_t2", 4, "PSUM")
    transpose_block(mbf, mT, NQ, NQ, ptp2)
    close(ptp2, mbfp)
    if last_stage < 7:
        return

    # ---------------- S7: fc + gelu --------------------------------------
    gtp = pool("gT", 1, side="right")
    gT = gtp.tile([128, DFF // 128, K], FP8 if FP8_PROJ else BF16)
    wfp = pool("wfc", 3)
    psf = pool("psum_f", 4, "PSUM")
    NQT = DFF // 8
    DR = mybir.MatmulPerfMode.DoubleRow
    for qtr in range(8):
        if FP8_FC:
            wf = wfp.tile([128, C // 256, 2, NQT], FP8, tag="wf")
            nc.sync.dma_start(
                out=wf[:],
                in_=io["wfcT8"].ap()[:, :, :, qtr * NQT:(qtr + 1) * NQT]
                .rearrange("c i p f -> p c i f"))
        else:
            wf = wfp.tile([128, NQ, NQT], BF16, tag="wf")
            nc.sync.dma_start(
                out=wf[:],
                in_=io["wfcT"].ap()[:, qtr * NQT:(qtr + 1) * NQT]
                .rearrange("(cc p) f -> p cc f", p=128))
        for fm in range(NQT // 128):
            ffm = qtr * (NQT // 128) + fm
            for nt in range(2):
                ps = psf.tile([128, 512], F32, tag="ps_f")
                if FP8_FC:
                    for c2 in range(C // 256):
                        nc.tensor.matmul(out=ps[:],
                                         lhsT=wf[:, c2, :, fm * 128:(fm + 1) * 128],
                                         rhs=mT[:, 2 * c2:2 * c2 + 2,
                                                nt * 512:(nt + 1) * 512],
                                         start=(c2 == 0),
                                         stop=(c2 == C // 256 - 1), perf_mode=DR)
                    nc.scalar.activation(out=gT[:, ffm, nt * 512:(nt + 1) * 512],
                                         in_=ps[:], func=Act.Gelu,
                                         scale=1.0 / WSCALE)
                else:
                    for cc in range(NQ):
                        nc.tensor.matmul(out=ps[:], lhsT=wf[:, cc, fm * 128:(fm + 1) * 128],
                                         rhs=mT[:, cc, nt * 512:(nt + 1) * 512],
                                         start=(cc == 0), stop=(cc == NQ - 1))
                    nc.scalar.activation(out=gT[:, ffm, nt * 512:(nt + 1) * 512],
                                         in_=ps[:], func=Act.Gelu)
    close(psf, wfp, mtp, lnp, cbp)
    dump("gT", gT[:])
    if last_stage < 8:
        return

    # ---------------- S8: proj + delta = proj_out + dsb (in place) --------
    delta = dsb
    psp = pool("psum_p", 4, "PSUM")
    if FP8_PROJ:
        wpp = pool("wproj", 1)
        wp8 = wpp.tile([128, DFF // 256, 2, C], FP8)
        nc.sync.dma_start(out=wp8[:],
                          in_=io["wprojT8"].ap().rearrange("f i p c -> p f i c"))
        for tt in range(NQ):
            for nt in range(2):
                ps = psp.tile([128, 512], F32, tag="ps_p")
                for f2 in range(DFF // 256):
                    nc.tensor.matmul(out=ps[:],
                                     lhsT=gT[:, 2 * f2:2 * f2 + 2,
                                             tt * 128:(tt + 1) * 128],
                                     rhs=wp8[:, f2, :, nt * 512:(nt + 1) * 512],
                                     start=(f2 == 0), stop=(f2 == DFF // 256 - 1),
                                     perf_mode=DR)
                sl = slice(nt * 512, (nt + 1) * 512)
                nc.vector.scalar_tensor_tensor(out=delta[:, tt, sl], in0=ps[:],
                                               scalar=1.0 / WSCALE,
                                               in1=dsb[:, tt, sl],
                                               op0=Alu.mult, op1=Alu.add)
    else:
        wpp = pool("wproj", 1)
        wps = []
        for half in range(2):
            wp = wpp.tile([128, DFF // 256, C], BF16, name=f"wp{half}")
            nc.sync.dma_start(
                out=wp[:],
                in_=io["wprojT"].ap()[half * DFF // 2:(half + 1) * DFF // 2, :]
                .rearrange("(fc p) c -> p fc c", p=128))
            wps.append(wp)
        for tt in range(NQ):
            for nt in range(2):
                ps = psp.tile([128, 512], F32, tag="ps_p")
                for fc2 in range(DFF // 128):
                    wp = wps[fc2 // (DFF // 256)]
                    fm = fc2 % (DFF // 256)
                    nc.tensor.matmul(out=ps[:], lhsT=gT[:, fc2, tt * 128:(tt + 1) * 128],
                                     rhs=wp[:, fm, nt * 512:(nt + 1) * 512],
                                     start=(fc2 == 0), stop=(fc2 == DFF // 128 - 1))
                sl = slice(nt * 512, (nt + 1) * 512)
                nc.vector.scalar_tensor_tensor(out=delta[:, tt, sl], in0=ps[:],
                                               scalar=1.0, in1=dsb[:, tt, sl],
                                               op0=Alu.mult, op1=Alu.add)
    close(psp, wpp, gtp)
    dump("delta", delta[:])
    if last_stage < 9:
        return

    # ---------------- S9: scatter matmul + combine + store ----------------
    xqp = pool("xq", 3)
    fin = pool("fin", 2)
    ps_sc = pool("psum_sc", 2, "PSUM")
    outr = out.rearrange("(c p) d -> p c d", p=128)

    def scatter_tile(tc, xq):
        ps = ps_sc.tile([128, C], F32, tag="ps_sc")
        for nt in range(2):
            for kc in range(NQ):
                nc.tensor.matmul(out=ps[:, nt * 512:(nt + 1) * 512],
                                 lhsT=pw[:, kc, tc * 128:(tc + 1) * 128],
                                 rhs=delta[:, kc, nt * 512:(nt + 1) * 512],
                                 start=(kc == 0), stop=(kc == NQ - 1))
        res = fin.tile([128, C], F32, tag="res")
        for nt in range(2):
            sl = slice(nt * 512, (nt + 1) * 512)
            nc.vector.scalar_tensor_tensor(out=res[:, sl], in0=xq[:, sl],
                                           scalar=wm1[:, tc:tc + 1], in1=ps[:, sl],
                                           op0=Alu.mult, op1=Alu.add)
        nc.sync.dma_start(out=outr[:, tc, :], in_=res[:])

    xqs = []
    for tc in range(NCH):
        xq = xqp.tile([128, C], F32, tag="xq")
        nc.sync.dma_start(out=xq[:], in_=xbr[:, tc, :])
        xqs.append(xq)
        if tc >= 2:
            scatter_tile(tc - 2, xqs[tc - 2])
    scatter_tile(NCH - 2, xqs[NCH - 2])
    scatter_tile(NCH - 1, xqs[NCH - 1])
    close(ps_sc, fin, xqp, dsp, pwp)


_CACHED = {}


def _get_program(dbg_names=(), last_stage=99):
    key = (tuple(dbg_names), last_stage)
    if key not in _CACHED:
        nc = bass.Bass("TRN2", target_bir_lowering=False, debug=False)
        io, dbg = declare_io(nc, dbg_names)
        with FunnelTileContext(nc) as tc:
            build(nc, tc, io, dbg, last_stage=last_stage)
        fix_sync_waits(nc)
        _CACHED[key] = nc
    return _CACHED[key]


def kernel(**inputs) -> np.ndarray:
    nc = _get_program()
    in_maps = host_inputs(inputs)
    res = run_bass_kernel_spmd(nc, in_maps, core_ids=list(range(B)))
    return np.stack([np.asarray(res.results[b]["out"], np.float32)
                     for b in range(B)])
